# revision 4
# baseline (speedup 1.0000x reference)
"""Navier-Stokes PINN kernel for 8x Trainium2 NeuronCores.

Math: the reference MLP uses ReLU activations, so the network is piecewise
linear in its inputs; all second derivatives vanish and the PDE residuals
collapse to
    u = dpsi/dy,  v = -dpsi/dx,  p = MLP(z)[1],  f = dp/dx,  g = dp/dy.
Everything comes from one forward pass plus two forward-mode tangent streams
(d/dx, d/dy) through the masked linear layers, with the 5 outputs produced by
one accumulated matmul against a host-assembled Wfin.

Numerics: matmuls run in float32r (single-pass, ~11 mantissa bits, 4x faster
than fp32 on the PE). The ReLU masks must match the fp32 reference masks to
~2^-15 relative accuracy, which a single f32r pass cannot deliver, so the
forward stream uses a 3-product compensated scheme per weight/activation
pair:  W^T H  ~=  Whi~Hhi + Whi~Hres + Wres~Hhi  (hi = f32r rounding,
res = exact residual, rounded), leaving a ~2^-22 error. Tangent streams and
the output projection are plain single-pass f32r.

Layout: feature-major — activations are (features, points). The two 128-wide
feature halves of each 256-wide layer live side by side in one (128, 2B)
tile ([h0 | h1] along the free axis), so relu/round/residual/mask are one
wide instruction each instead of four. Tangent x/y directions share a
(128, 2B) tile per half the same way.

Engine budget per 512-point block (ns, busy): PE 19630 (56 matmul-equiv
passes), Act ~13100 (relu, hi-rounding, 2 masks, out-bias), DVE ~13900
(tangent gating, tangent init, 3 masks... see _build), Pool ~8700
(residuals). PSUM: A ring 1x2 banks, T ring 2x2, out ring 2x1 = 8 banks.

Sharding: pure data parallel, 8192 points per core, weights replicated.
"""

import numpy as np

NCORES = 8
N_TOTAL = 65536
NPC = N_TOTAL // NCORES  # points per core
HID = 256
NL = 4  # hidden->hidden layers (L=6 total: in + 4 hidden + out)
B = 512  # points per block
NB = NPC // B
P = 128
NH = HID // P  # feature halves

_NC_CACHE = {}


def _build(zero_bias: bool):
    import concourse.tile as tile
    from concourse import bacc, mybir

    f32 = mybir.dt.float32
    f32r = mybir.dt.float32r
    Relu = mybir.ActivationFunctionType.Relu
    Ident = mybir.ActivationFunctionType.Identity
    Copy = mybir.ActivationFunctionType.Copy
    mult = mybir.AluOpType.mult
    sub = mybir.AluOpType.subtract
    is_gt = mybir.AluOpType.is_gt

    nc = bacc.Bacc(
        "TRN2",
        target_bir_lowering=False,
        debug=False,
        enable_asserts=False,
        num_devices=NCORES,
    )

    zt_d = nc.dram_tensor("zt", (3, NPC), f32, kind="ExternalInput").ap()
    win_d = nc.dram_tensor("win", (3, HID), f32, kind="ExternalInput").ap()
    wint_d = nc.dram_tensor("wint", (HID, 2), f32, kind="ExternalInput").ap()
    bin_d = nc.dram_tensor("bin", (HID, 1), f32, kind="ExternalInput").ap()
    wh_d = nc.dram_tensor("wh", (NL, HID, HID), f32, kind="ExternalInput").ap()
    bh_d = nc.dram_tensor("bh", (NL, HID, 1), f32, kind="ExternalInput").ap()
    wfin_d = nc.dram_tensor("wfin", (3 * HID, 5), f32, kind="ExternalInput").ap()
    bfin_d = nc.dram_tensor("bfin", (5, 1), f32, kind="ExternalInput").ap()
    out_d = nc.dram_tensor("out", (5, NPC), f32, kind="ExternalOutput").ap()

    with tile.TileContext(nc) as tc:
        with (
            tc.tile_pool(name="weights", bufs=1) as wpool,
            tc.tile_pool(name="zin", bufs=3) as zpool,
            tc.tile_pool(name="acts", bufs=2) as hpool,
            tc.tile_pool(name="tans", bufs=2) as gpool,
            tc.tile_pool(name="masks", bufs=2) as mpool,
            tc.tile_pool(name="outs", bufs=2) as opool,
            tc.tile_pool(name="ps", bufs=1, space="PSUM") as ps,
        ):
            # ---- one-time weight staging (hi = f32r rounding, res = W - hi) ----
            def stage_pair(name, shape, src_ap):
                t = wpool.tile(shape, f32, tag=name, name=name)
                nc.sync.dma_start(t[:], src_ap)
                hi = wpool.tile(shape, f32, tag=name + "h", name=name + "h")
                nc.scalar.activation(hi[:].bitcast(f32r), t[:], Copy)
                rs = wpool.tile(shape, f32, tag=name + "s", name=name + "s")
                nc.vector.tensor_tensor(rs[:].bitcast(f32r), t[:], hi[:], sub)
                return hi, rs

            win_hi, win_rs = stage_pair("win", [3, HID], win_d[:, :])
            wint_t = []
            bin_t = []
            for h in range(NH):
                w = wpool.tile([P, 2], f32, tag=f"wint{h}", name=f"wint{h}")
                nc.sync.dma_start(w[:], wint_d[h * P : (h + 1) * P, :])
                wint_t.append(w)
                if not zero_bias:
                    b = wpool.tile([P, 1], f32, tag=f"bin{h}", name=f"bin{h}")
                    nc.sync.dma_start(b[:], bin_d[h * P : (h + 1) * P, :])
                    bin_t.append(b)
            wh_hi = {}
            wh_rs = {}
            bh_t = {}
            for li in range(NL):
                for k in range(NH):
                    for h in range(NH):
                        wh_hi[li, k, h], wh_rs[li, k, h] = stage_pair(
                            f"wh{li}{k}{h}",
                            [P, P],
                            wh_d[li, k * P : (k + 1) * P, h * P : (h + 1) * P],
                        )
                if not zero_bias:
                    for h in range(NH):
                        b = wpool.tile([P, 1], f32, tag=f"bh{li}{h}", name=f"bh{li}{h}")
                        nc.sync.dma_start(b[:], bh_d[li, h * P : (h + 1) * P, :])
                        bh_t[li, h] = b
            wfin_t = []
            for k in range(3 * NH):
                w = wpool.tile([P, 5], f32, tag=f"wfin{k}", name=f"wfin{k}")
                nc.sync.dma_start(w[:], wfin_d[k * P : (k + 1) * P, :])
                wr = wpool.tile([P, 5], f32, tag=f"wfin{k}r", name=f"wfin{k}r")
                nc.scalar.activation(wr[:].bitcast(f32r), w[:], Copy)
                wfin_t.append(wr)
            bfin_t = wpool.tile([5, 1], f32, tag="bfin", name="bfin")
            nc.sync.dma_start(bfin_t[:], bfin_d[:, :])

            # ---- per-block pipeline ----
            # mask engine split: Act layers {0, 2}, Pool {1, 3}, DVE {4}.
            def make_mask(mt_ap, h_ap, idx):
                eng = (nc.scalar, nc.gpsimd, nc.scalar, nc.gpsimd, nc.vector)[idx]
                if eng is nc.scalar:
                    nc.scalar.sign(mt_ap, h_ap)
                else:
                    eng.tensor_scalar(mt_ap, h_ap, 0.0, None, is_gt)

            for ib in range(NB):
                zt = zpool.tile([3, B], f32, tag="zt", name="zt")
                nc.sync.dma_start(zt[:], zt_d[:, ib * B : (ib + 1) * B])
                zhi = zpool.tile([3, B], f32, tag="zhi", name="zhi")
                nc.scalar.activation(zhi[:].bitcast(f32r), zt[:], Copy)
                zrs = zpool.tile([3, B], f32, tag="zrs", name="zrs")
                nc.vector.tensor_tensor(zrs[:].bitcast(f32r), zt[:], zhi[:], sub)

                # input layer: A1 = Win.T @ z, 3-product compensated
                a = ps.tile([P, 2 * B], f32, tag="A", name="A", bufs=1)
                for h in range(NH):
                    asl = a[:, h * B : (h + 1) * B]
                    whi = win_hi[:].bitcast(f32r)[:, h * P : (h + 1) * P]
                    wrs = win_rs[:].bitcast(f32r)[:, h * P : (h + 1) * P]
                    prods = [
                        (whi, zhi[:].bitcast(f32r)),
                        (whi, zrs[:].bitcast(f32r)),
                        (wrs, zhi[:].bitcast(f32r)),
                    ]
                    for i, (lhs, rhs) in enumerate(prods):
                        nc.tensor.matmul(
                            asl, lhs, rhs, start=(i == 0), stop=(i == len(prods) - 1)
                        )

                def relu_pair(a, li, last):
                    """relu + f32r hi/res pair + mask from merged PSUM A.

                    li: 0-based layer index (0 = input layer) for bias lookup
                    and mask-engine round-robin. last: final hidden layer —
                    rounded-only H (feeds the f32r out-matmul), no residual.
                    """
                    ht = hpool.tile([P, 2 * B], f32, tag="H", name="H")
                    hdst = ht[:].bitcast(f32r) if last else ht[:]
                    if zero_bias:
                        nc.scalar.activation(hdst, a[:], Relu)
                    else:
                        bt = bin_t if li == 0 else [bh_t[li - 1, 0], bh_t[li - 1, 1]]
                        for h in range(NH):
                            nc.scalar.activation(
                                hdst[:, h * B : (h + 1) * B],
                                a[:, h * B : (h + 1) * B],
                                Relu,
                                bias=bt[h][:, 0:1],
                            )
                    mt = mpool.tile([P, 2 * B], f32, tag="M", name="M")
                    make_mask(mt[:], ht[:], li)
                    if last:
                        return ht, None, mt
                    hhi = hpool.tile([P, 2 * B], f32, tag="Hh", name="Hh")
                    nc.scalar.activation(hhi[:].bitcast(f32r), ht[:], Copy)
                    hrs = hpool.tile([P, 2 * B], f32, tag="Hs", name="Hs")
                    nc.gpsimd.tensor_tensor(hrs[:].bitcast(f32r), ht[:], hhi[:], sub)
                    return hhi, hrs, mt

                hhi, hrs, mt = relu_pair(a, 0, last=False)

                # input tangent init: G1 = M1 * Win-row (per half, [x|y] merged)
                Gs = []
                for h in range(NH):
                    gt = gpool.tile([P, 2 * B], f32, tag=f"G{h}", name=f"G{h}")
                    for d in range(2):
                        nc.vector.tensor_scalar(
                            gt[:, d * B : (d + 1) * B].bitcast(f32r),
                            mt[:, h * B : (h + 1) * B],
                            wint_t[h][:, d : d + 1],
                            None,
                            mult,
                        )
                    Gs.append(gt)

                # hidden layers
                for li in range(NL):
                    a = ps.tile([P, 2 * B], f32, tag="A", name="A", bufs=1)
                    for h in range(NH):
                        asl = a[:, h * B : (h + 1) * B]
                        i = 0
                        for k in range(NH):
                            whi = wh_hi[li, k, h][:].bitcast(f32r)
                            wrs = wh_rs[li, k, h][:].bitcast(f32r)
                            hh = hhi[:].bitcast(f32r)[:, k * B : (k + 1) * B]
                            hs = hrs[:].bitcast(f32r)[:, k * B : (k + 1) * B]
                            for lhs, rhs in ((whi, hh), (whi, hs), (wrs, hh)):
                                nc.tensor.matmul(
                                    asl, lhs, rhs, start=(i == 0), stop=(i == 6 - 1)
                                )
                                i += 1
                    tps = []
                    for h in range(NH):
                        tp = ps.tile([P, 2 * B], f32, tag="T", name="T", bufs=2)
                        for d in range(2):
                            for k in range(NH):
                                nc.tensor.matmul(
                                    tp[:, d * B : (d + 1) * B],
                                    wh_hi[li, k, h][:].bitcast(f32r),
                                    Gs[k][:, d * B : (d + 1) * B].bitcast(f32r),
                                    start=(k == 0),
                                    stop=(k == NH - 1),
                                )
                        tps.append(tp)
                    hhi, hrs, mt = relu_pair(a, li + 1, last=(li == NL - 1))
                    nGs = []
                    for h in range(NH):
                        gt = gpool.tile([P, 2 * B], f32, tag=f"G{h}", name=f"G{h}")
                        m3 = (
                            mt[:, h * B : (h + 1) * B]
                            .unsqueeze(1)
                            .broadcast_to((P, 2, B))
                        )
                        nc.vector.tensor_tensor(
                            gt[:].bitcast(f32r).rearrange("p (d b) -> p d b", d=2),
                            tps[h][:].rearrange("p (d b) -> p d b", d=2),
                            m3,
                            mult,
                        )
                        nGs.append(gt)
                    Gs = nGs

                # output layer: [H ; Gx ; Gy] @ Wfin -> (5, B)
                ops = ps.tile([5, B], f32, tag="O", name="O", bufs=2)
                chunks = [hhi[:].bitcast(f32r)[:, 0:B], hhi[:].bitcast(f32r)[:, B : 2 * B]]
                for d in range(2):
                    for h in range(NH):
                        chunks.append(
                            Gs[h][:, d * B : (d + 1) * B].bitcast(f32r)
                        )
                for k in range(6):
                    nc.tensor.matmul(
                        ops[:],
                        wfin_t[k][:].bitcast(f32r),
                        chunks[k],
                        start=(k == 0),
                        stop=(k == 5),
                    )
                osb = opool.tile([5, B], f32, tag="osb", name="osb")
                nc.scalar.activation(osb[:], ops[:], Ident, bias=bfin_t[:, 0:1])
                nc.scalar.dma_start(out_d[:, ib * B : (ib + 1) * B], osb[:])

    nc.compile()
    return nc


def _get_nc(zero_bias=True):
    if zero_bias not in _NC_CACHE:
        _NC_CACHE[zero_bias] = _build(zero_bias)
    return _NC_CACHE[zero_bias]


def kernel(x, y, t, Win, b_in, Wh, b_h, Wout, b_out, _trace=False):
    from concourse import bass_utils

    x = np.asarray(x, np.float32)
    y = np.asarray(y, np.float32)
    t = np.asarray(t, np.float32)
    Win = np.asarray(Win, np.float32)
    b_in = np.asarray(b_in, np.float32)
    Wh = np.asarray(Wh, np.float32)
    b_h = np.asarray(b_h, np.float32)
    Wout = np.asarray(Wout, np.float32)
    b_out = np.asarray(b_out, np.float32)

    z = np.ascontiguousarray(
        np.stack([x[:, 0], y[:, 0], t[:, 0]], axis=0)
    )  # (3, N)
    wint = np.ascontiguousarray(Win[0:2, :].T)  # (HID, 2)
    binc = np.ascontiguousarray(b_in.reshape(HID, 1))
    bhc = np.ascontiguousarray(b_h.reshape(NL, HID, 1))
    wfin = np.zeros((3 * HID, 5), np.float32)
    wfin[2 * HID : 3 * HID, 0] = Wout[:, 0]  # u = dpsi/dy
    wfin[HID : 2 * HID, 1] = -Wout[:, 0]  # v = -dpsi/dx
    wfin[0:HID, 2] = Wout[:, 1]  # p
    wfin[HID : 2 * HID, 3] = Wout[:, 1]  # f = dp/dx
    wfin[2 * HID : 3 * HID, 4] = Wout[:, 1]  # g = dp/dy
    bfin = np.zeros((5, 1), np.float32)
    bfin[2, 0] = b_out[1]

    zero_bias = not (np.any(b_in) or np.any(b_h))
    nc = _get_nc(zero_bias)
    in_maps = []
    for c in range(NCORES):
        in_maps.append(
            {
                "zt": np.ascontiguousarray(z[:, c * NPC : (c + 1) * NPC]),
                "win": Win,
                "wint": wint,
                "bin": binc,
                "wh": Wh,
                "bh": bhc,
                "wfin": wfin,
                "bfin": bfin,
            }
        )
    res = bass_utils.run_bass_kernel_spmd(
        nc, in_maps, core_ids=list(range(NCORES)), trace=_trace
    )
    kernel._last_results = res
    full = np.concatenate(
        [res.results[c]["out"] for c in range(NCORES)], axis=1
    )  # (5, N)
    return np.ascontiguousarray(full[:, :, None].astype(np.float32))
